# revision 1
# baseline (speedup 1.0000x reference)
"""Trainium2 Bass kernel for Restormer-style transposed (channel) attention.

Per-core (1 of 8 batch elements):
  qkv = W_qkv @ x            (PE, fp32r; q/k channels host-permuted into per-head
                              packed chunks, staged per 16-row slab)
  qkv = dwconv3x3(qkv)       (PE: 9 accumulated diag-matmuls over a zero-padded
                              row layout; the diag matrices also scatter q/k
                              channels into 32-aligned per-head slots for free)
  q,k -> bf16 -> PE-transpose into qki [px, slot] tiles
       -> gram G accumulated in PSUM over all pixels (PE, bf16)
  v   -> HBM scratch (fp32r)
  softmax over normalized gram; fold proj: M = W_proj @ blockdiag(A)
  out = M @ v                (PE, fp32r)
"""
import numpy as np

NUM_HEADS = 8
C = 192
H = W = 128
HW = H * W
C3 = 3 * C            # 576
CD = C // NUM_HEADS   # 24
NCORES = 8
SLAB = 16
NSLABS = H // SLAB
EPS = 1e-12
PW = W + 4            # padded row width (2 zero cols each side)
IMG0 = 2              # image column offset within a padded row

# chunk table: (wqT column base, input channels, dw output partitions, is_qk)
# j<4: packed q/k channels of heads {2j, 2j+1}; j>=4: v channels (plain order)
CHUNKS = [(0, 96, 128, True), (96, 96, 128, True), (192, 96, 128, True),
          (288, 96, 128, True), (384, 128, 128, False), (512, 64, 64, False)]
TAPS = [(0, 0)] + [(dy, dx) for dy in (-1, 0, 1) for dx in (-1, 0, 1) if (dy, dx) != (0, 0)]
NBLK = len(CHUNKS) * len(TAPS)          # 54 diag blocks


def _packed_channels(j):
    """Global qkv channel list for qk chunk j and their 32-aligned local slots."""
    chs, slots = [], []
    for b, (lo, n) in enumerate(((48 * j, 24), (192 + 48 * j, 24),
                                 (48 * j + 24, 24), (192 + 48 * j + 24, 24))):
        chs.extend(range(lo, lo + n))
        slots.extend(range(32 * b, 32 * b + n))
    return chs, slots


_CACHE = {}


def _build(reps=1):
    import concourse.bass as bass
    import concourse.mybir as mybir
    import concourse.tile as tile
    from concourse import bacc
    from contextlib import ExitStack

    dt = mybir.dt
    A = mybir.AluOpType
    AF = mybir.ActivationFunctionType
    AX = mybir.AxisListType
    f32, bf16, f32r = dt.float32, dt.bfloat16, dt.float32r

    nc = bacc.Bacc("TRN2", num_devices=NCORES)

    xd = nc.dram_tensor("x", [C, HW], f32r, kind="ExternalInput").ap()
    wqTd = nc.dram_tensor("wqT", [C, C3], f32r, kind="ExternalInput").ap()
    wpTd = nc.dram_tensor("wpT", [C, C], f32, kind="ExternalInput").ap()
    dgd = nc.dram_tensor("diagw", [128, NBLK * 128], f32r, kind="ExternalInput").ap()
    mskd = nc.dram_tensor("gmask", [128, 512], f32, kind="ExternalInput").ap()
    zrd = nc.dram_tensor("zeros", [128, 64], f32r, kind="ExternalInput").ap()
    tmpd = nc.dram_tensor("tmap", [128, 4], f32, kind="ExternalInput").ap()
    eyed = nc.dram_tensor("eye", [128, 128], f32, kind="ExternalInput").ap()
    eyebd = nc.dram_tensor("eyeb", [128, 128], bf16, kind="ExternalInput").ap()
    outd = nc.dram_tensor("out", [C, HW], f32, kind="ExternalOutput").ap()

    with tile.TileContext(nc) as tc:
      with ExitStack() as _es:
        cpool = _es.enter_context(tc.tile_pool(name="const", bufs=1))
        xpool = _es.enter_context(tc.tile_pool(name="xin", bufs=1))
        spool = _es.enter_context(tc.tile_pool(name="stage", bufs=1))
        bpool = _es.enter_context(tc.tile_pool(name="qkbf", bufs=3))
        qpool = _es.enter_context(tc.tile_pool(name="qki", bufs=68))
        vpool = _es.enter_context(tc.tile_pool(name="vst", bufs=1))
        vlpool = _es.enter_context(tc.tile_pool(name="vld", bufs=4))
        mpool = _es.enter_context(tc.tile_pool(name="sm", bufs=2))
        apool = _es.enter_context(tc.tile_pool(name="abd", bufs=1))
        opool = _es.enter_context(tc.tile_pool(name="outs", bufs=2))
        dpool = _es.enter_context(tc.tile_pool(name="dram", bufs=1, space="DRAM"))
        psA = _es.enter_context(tc.tile_pool(name="psA", bufs=2, space="PSUM"))
        psB = _es.enter_context(tc.tile_pool(name="psB", bufs=2, space="PSUM"))
        psG = _es.enter_context(tc.tile_pool(name="psG", bufs=1, space="PSUM"))
        psT = _es.enter_context(tc.tile_pool(name="psT", bufs=3, space="PSUM"))
        if True:
            # ---------- constants ----------
            # all constants on the ACT hwdge queue, ordered by first use,
            # so the sync queue carries only x-slab streaming from t=0
            wq0 = cpool.tile([128, C3], f32r, tag="wq0")
            nc.scalar.dma_start(wq0[:, :], wqTd[0:128, :])
            wq1 = cpool.tile([64, C3], f32r, tag="wq1")
            nc.scalar.dma_start(wq1[:, :], wqTd[128:192, :])
            zc = cpool.tile([128, 64], f32r, tag="zc")
            nc.scalar.dma_start(zc[:, :], zrd[:, :])
            dg = cpool.tile([128, NBLK * 128], f32r, tag="dg")
            half = (NBLK // 2) * 128
            nc.scalar.dma_start(dg[:, 0:half], dgd[:, 0:half])
            nc.scalar.dma_start(dg[:, half:], dgd[:, half:])
            eyeb = cpool.tile([128, 128], bf16, tag="eyeb")
            nc.scalar.dma_start(eyeb[:, :], eyebd[:, :])
            msk = cpool.tile([128, 512], f32, tag="msk")
            nc.scalar.dma_start(msk[:, :], mskd[:, :])
            tmap = cpool.tile([128, 4], f32, tag="tmap")
            nc.scalar.dma_start(tmap[:, :], tmpd[:, :])
            eye = cpool.tile([128, 128], f32, tag="eye")
            nc.scalar.dma_start(eye[:, :], eyed[:, :])
            wp0 = cpool.tile([96, C], f32, tag="wp0")
            nc.scalar.dma_start(wp0[:, :], wpTd[0:96, :])
            wp1 = cpool.tile([96, C], f32, tag="wp1")
            nc.scalar.dma_start(wp1[:, :], wpTd[96:192, :])

            vtmp = dpool.tile([C, HW], f32r, tag="vtmp")
            gram = psG.tile([128, 512], f32, tag="g")

            for _rep in range(reps):
                # ---------- pass 1 ----------
                qki_tiles = {}
                for s in range(NSLABS):
                    row_lo = max(0, SLAB * s - 1)
                    row_hi = min(H - 1, SLAB * s + SLAB)  # inclusive
                    nrows = row_hi - row_lo + 1
                    ncols = nrows * W
                    col0 = row_lo * W
                    xs0 = xpool.tile([128, ncols], f32r, tag="xs0")
                    xs1 = xpool.tile([64, ncols], f32r, tag="xs1")
                    nc.sync.dma_start(xs0[:, :], xd[0:128, col0:col0 + ncols])
                    nc.sync.dma_start(xs1[:, :], xd[128:192, col0:col0 + ncols])

                    stg = []
                    for j, (cb, nch, mout, isqk) in enumerate(CHUNKS):
                        st = spool.tile([nch, nrows * PW], f32r, tag=f"st{j}")
                        stg.append(st)
                        # zero the pad columns (ACT copies from a zeros tile)
                        stv = st[:, :].rearrange("p (r w) -> p r w", w=PW)
                        zv = zc[0:nch, 0:2 * nrows].rearrange("p (r w) -> p r w", w=2)
                        nc.scalar.copy(stv[:, :, 0:2], zv)
                        nc.scalar.copy(stv[:, :, W + 2:W + 4], zv)
                        tws = []
                        rem = ncols
                        while rem > 0:
                            t = min(512, rem)
                            if rem - t == 128:
                                t = 384      # keep every fp32r moving >= 256
                            tws.append(t)
                            rem -= t
                        t0 = 0
                        for ti, tw in enumerate(tws):
                            rr = t0 // W
                            nr4 = tw // W
                            ps = psA.tile([nch, tw], f32, tag="qkv")
                            nc.tensor.matmul(ps[:, :], wq0[:, cb:cb + nch],
                                             xs0[:, t0:t0 + tw], start=True, stop=False)
                            nc.tensor.matmul(ps[:, :], wq1[:, cb:cb + nch],
                                             xs1[:, t0:t0 + tw], start=False, stop=True)
                            ev = nc.vector.tensor_copy if ti % 2 == 0 else nc.scalar.copy
                            ev(stv[:, rr:rr + nr4, IMG0:IMG0 + W],
                               ps[:, :].rearrange("p (r w) -> p r w", w=W))
                            t0 += tw

                    DWT = [(0, 3), (3, 3), (6, 3), (9, 3), (12, 2), (14, 2)]
                    for j, (cb, nch, mout, isqk) in enumerate(CHUNKS):
                        st = stg[j]
                        qbs = {}
                        vs = None
                        if not isqk:
                            vs = vpool.tile([mout, SLAB * W], f32r, tag=f"vs{j}")
                        for (lt0, nrt) in DWT:
                            R0 = SLAB * s + lt0
                            pd = psB.tile([mout, nrt * PW], f32, tag="dw")
                            for k, (dy, dx) in enumerate(TAPS):
                                rlo = max(R0, -dy)
                                rhi = min(R0 + nrt - 1, H - 1 - dy)
                                nr = rhi - rlo + 1
                                L = nr * PW - 4
                                mo = (rlo + dy - row_lo) * PW + IMG0 + dx
                                oo = (rlo - R0) * PW + IMG0
                                blk = (9 * j + k) * 128
                                nc.tensor.matmul(
                                    pd[:, oo:oo + L],
                                    dg[0:nch, blk:blk + mout],
                                    st[:, mo:mo + L],
                                    start=(k == 0), stop=(k == len(TAPS) - 1))
                            pdv = pd[:, :].rearrange("p (r w) -> p r w", w=PW)
                            if isqk:
                                # copy rows into 4-row qb groups (split at edges)
                                r = 0
                                while r < nrt:
                                    gidx = (lt0 + r) // 4
                                    n_in = min(nrt - r, 4 - (lt0 + r) % 4)
                                    qb = qbs.get(gidx)
                                    if qb is None:
                                        qb = bpool.tile([128, 512], bf16, tag="qkbf")
                                        qbs[gidx] = qb
                                    qrow = (lt0 + r) % 4
                                    ev3 = (nc.scalar.copy if (gidx + j) % 2 == 0
                                           else nc.vector.tensor_copy)
                                    ev3(qb[:, :].rearrange("p (r w) -> p r w", w=W)[
                                            :, qrow:qrow + n_in, :],
                                        pdv[:, r:r + n_in, IMG0:IMG0 + W])
                                    r += n_in
                                    if qrow + n_in == 4:
                                        for u in range(4):
                                            g = SLAB * s + 4 * gidx + u
                                            qi = qki_tiles.get(g)
                                            if qi is None:
                                                qi = []
                                                for _q4 in range(4):
                                                    qit = qpool.tile([128, 128],
                                                                     bf16, tag="qki")
                                                    qi.append(qit)
                                                qki_tiles[g] = qi
                                            ptb = psT.tile([128, 128], bf16,
                                                           tag="tr")
                                            nc.tensor.transpose(
                                                ptb[:, :],
                                                qb[:, 128 * u:128 * (u + 1)],
                                                eyeb[:, :])
                                            ev2 = (nc.vector.tensor_copy
                                                   if (g + j) % 2 == 0
                                                   else nc.scalar.copy)
                                            ev2(qi[j][:, :], ptb[:, :])
                                else:
                                    pass
                            else:
                                nc.vector.tensor_copy(
                                    vs[:, lt0 * W:(lt0 + nrt) * W].rearrange(
                                        "p (r w) -> p r w", w=W),
                                    pdv[:, :, IMG0:IMG0 + W])
                        if not isqk:
                            vr0 = cb - 384
                            nc.sync.dma_start(
                                vtmp[vr0:vr0 + mout,
                                     SLAB * s * W:(SLAB * s + SLAB) * W],
                                vs[:, :])

                    # gram updates for this slab's px-subtiles
                    for u in range(SLAB):
                        g = SLAB * s + u
                        qi = qki_tiles.pop(g)
                        for p in range(4):
                            nc.tensor.matmul(
                                gram[:, 128 * p:128 * (p + 1)],
                                qi[p][:, :],
                                qi[p][:, :],
                                start=(g == 0), stop=(g == H - 1),
                                skip_group_check=True)

                # ---------- norms ----------
                gm = mpool.tile([128, 512], f32, tag="gm")
                nc.vector.tensor_tensor(gm[:, :], gram[:, :], msk[:, :], A.mult)
                s_sb = mpool.tile([128, 4], f32, tag="ssb")
                nc.vector.tensor_reduce(s_sb[:, :],
                                        gm[:, :].rearrange("p (g c) -> p g c", g=4),
                                        AX.X, A.add)
                ns = mpool.tile([128, 4], f32, tag="ns")
                nc.scalar.sqrt(ns[:, :], s_sb[:, :])
                nsc = mpool.tile([128, 4], f32, tag="nsc")
                nc.vector.tensor_scalar_max(nsc[:, :], ns[:, :], EPS)
                ry = mpool.tile([128, 4], f32, tag="ry")
                nc.vector.reciprocal(ry[:, :], nsc[:, :])
                t1 = mpool.tile([128, 4], f32, tag="t1")
                nc.vector.tensor_tensor(t1[:, :], s_sb[:, :], ry[:, :], A.mult)
                t2 = mpool.tile([128, 4], f32, tag="t2")
                nc.vector.tensor_add(t2[:, :], nsc[:, :], t1[:, :])
                ns2 = mpool.tile([128, 4], f32, tag="ns2")
                nc.vector.tensor_scalar_mul(ns2[:, :], t2[:, :], 0.5)
                ns3 = mpool.tile([128, 4], f32, tag="ns3")
                nc.vector.tensor_scalar_max(ns3[:, :], ns2[:, :], EPS)
                rn = mpool.tile([128, 4], f32, tag="rn")
                nc.vector.reciprocal(rn[:, :], ns3[:, :])
                rkt = mpool.tile([128, 4], f32, tag="rkt")
                nc.vector.tensor_tensor(rkt[:, :], rn[:, :], tmap[:, :], A.mult)
                rq = mpool.tile([24, 8], f32, tag="rq")
                nc.sync.dma_start(rq[0:24, 0:7:2], rn[0:24, 0:4])
                nc.sync.dma_start(rq[0:24, 1:8:2], rn[64:88, 0:4])

                # ---------- softmax + A blockdiag ----------
                a0 = apool.tile([96, C], f32, tag="a0")
                a1 = apool.tile([96, C], f32, tag="a1")
                nc.vector.memset(a0[:, :], 0.0)
                nc.vector.memset(a1[:, :], 0.0)
                bt = mpool.tile([128, 8 * CD], f32, tag="bt")
                for h in range(NUM_HEADS):
                    p = h // 2
                    if h % 2 == 0:
                        kbase, qcol = 32, 0
                    else:
                        kbase, qcol = 96, 64
                    nc.vector.tensor_scalar_mul(
                        bt[kbase:kbase + CD, CD * h:CD * (h + 1)],
                        gram[kbase:kbase + CD, 128 * p + qcol:128 * p + qcol + CD],
                        rkt[kbase:kbase + CD, p:p + 1])
                    ptr = psA.tile([CD, CD], f32, tag="qkv")
                    nc.tensor.transpose(ptr[:, :],
                                        bt[kbase:kbase + CD, CD * h:CD * (h + 1)],
                                        eye[kbase:kbase + CD, kbase:kbase + CD],
                                        tile_position=(kbase, 0))
                    ls = mpool.tile([CD, CD], f32, tag="ls")
                    nc.vector.tensor_scalar_mul(ls[:, :], ptr[:, :], rq[0:24, h:h + 1])
                    mx = mpool.tile([CD, 1], f32, tag="mx")
                    nc.vector.tensor_reduce(mx[:, :], ls[:, :], AX.X, A.max)
                    mxn = mpool.tile([CD, 1], f32, tag="mxn")
                    nc.vector.tensor_scalar_mul(mxn[:, :], mx[:, :], -1.0)
                    es = mpool.tile([CD, CD], f32, tag="es")
                    se = mpool.tile([CD, 1], f32, tag="se")
                    nc.scalar.activation(es[:, :], ls[:, :], AF.Exp,
                                         bias=mxn[0:CD, 0:1], scale=1.0,
                                         accum_out=se[:, :])
                    rse = mpool.tile([CD, 1], f32, tag="rse")
                    nc.vector.reciprocal(rse[:, :], se[:, :])
                    ah = mpool.tile([CD, CD], f32, tag="ah")
                    nc.vector.tensor_scalar_mul(ah[:, :], es[:, :], rse[0:CD, 0:1])
                    adst = a0 if h < 4 else a1
                    r0 = 24 * (h % 4)
                    nc.sync.dma_start(adst[r0:r0 + CD, CD * h:CD * (h + 1)], ah[:, :])

                # ---------- M^T = A_bd^T @ W_proj^T ----------
                mt0 = cpool.tile([128, C], f32r, tag="mt0")
                mt1 = cpool.tile([64, C], f32r, tag="mt1")
                pmt0 = psA.tile([128, C], f32, tag="qkv")
                nc.tensor.matmul(pmt0[:, :], a0[:, 0:128], wp0[:, :],
                                 start=True, stop=False)
                nc.tensor.matmul(pmt0[:, :], a1[:, 0:128], wp1[:, :],
                                 start=False, stop=True)
                nc.scalar.copy(mt0[:, :], pmt0[:, :])
                pmt1 = psA.tile([64, C], f32, tag="qkv")
                nc.tensor.matmul(pmt1[:, :], a0[:, 128:192], wp0[:, :],
                                 start=True, stop=False)
                nc.tensor.matmul(pmt1[:, :], a1[:, 128:192], wp1[:, :],
                                 start=False, stop=True)
                nc.scalar.copy(mt1[:, :], pmt1[:, :])

                # ---------- pass 2: out = M @ v ----------
                BK = 1024
                for b0 in range(0, HW, BK):
                    vl0 = vlpool.tile([128, BK], f32r, tag="vl0")
                    vl1 = vlpool.tile([64, BK], f32r, tag="vl1")
                    nc.sync.dma_start(vl0[:, :], vtmp[0:128, b0:b0 + BK])
                    nc.sync.dma_start(vl1[:, :], vtmp[128:192, b0:b0 + BK])
                    oa = opool.tile([128, BK], f32, tag="oa")
                    ob = opool.tile([64, BK], f32, tag="ob")
                    for t0 in range(0, BK, 512):
                        pa = psA.tile([128, 512], f32, tag="qkv")
                        nc.tensor.matmul(pa[:, :], mt0[:, 0:128], vl0[:, t0:t0 + 512],
                                         start=True, stop=False)
                        nc.tensor.matmul(pa[:, :], mt1[:, 0:128], vl1[:, t0:t0 + 512],
                                         start=False, stop=True)
                        nc.scalar.copy(oa[:, t0:t0 + 512], pa[:, :])
                        pb = psB.tile([64, 512], f32, tag="dw")
                        nc.tensor.matmul(pb[:, :], mt0[:, 128:192], vl0[:, t0:t0 + 512],
                                         start=True, stop=False)
                        nc.tensor.matmul(pb[:, :], mt1[:, 128:192], vl1[:, t0:t0 + 512],
                                         start=False, stop=True)
                        nc.scalar.copy(ob[:, t0:t0 + 512], pb[:, :])
                    nc.scalar.dma_start(outd[0:128, b0:b0 + BK], oa[:, :])
                    nc.scalar.dma_start(outd[128:192, b0:b0 + BK], ob[:, :])

    nc.compile()
    return nc


def _host_consts(w_qkv, w_dw, w_proj, temperature):
    wqT_plain = np.ascontiguousarray(np.asarray(w_qkv, np.float32).T)   # [192, 576]
    wpT = np.ascontiguousarray(np.asarray(w_proj, np.float32).T)        # [192, 192]
    wd = np.asarray(w_dw, dtype=np.float32).reshape(C3, 3, 3)

    wqT = np.zeros_like(wqT_plain)
    dgw = np.zeros((128, NBLK * 128), dtype=np.float32)
    for j, (cb, nch, mout, isqk) in enumerate(CHUNKS):
        if isqk:
            chs, slots = _packed_channels(j)
            wqT[:, cb:cb + nch] = wqT_plain[:, chs]
            for k, (dy, dx) in enumerate(TAPS):
                col = (9 * j + k) * 128
                dgw[np.arange(nch), col + np.asarray(slots)] = \
                    wd[chs, dy + 1, dx + 1]
        else:
            gch = 384 + (cb - 384)
            wqT[:, cb:cb + nch] = wqT_plain[:, gch:gch + nch]
            for k, (dy, dx) in enumerate(TAPS):
                col = (9 * j + k) * 128
                idx = np.arange(nch)
                dgw[idx, col + idx] = wd[gch + idx, dy + 1, dx + 1]

    gmask = np.tile(np.eye(128, dtype=np.float32), (1, 4))              # [128, 512]
    zeros = np.zeros((128, 64), dtype=np.float32)
    tmapv = np.ones((128, 4), dtype=np.float32)
    tf = np.asarray(temperature, dtype=np.float32).reshape(-1)
    for p in range(4):
        tmapv[32:56, p] = tf[2 * p]
        tmapv[96:120, p] = tf[2 * p + 1]
    eyev = np.eye(128, dtype=np.float32)
    import ml_dtypes
    eyebv = np.eye(128).astype(ml_dtypes.bfloat16)
    return dict(wqT=wqT, wpT=wpT, diagw=dgw, gmask=gmask, tmap=tmapv,
                eye=eyev, eyeb=eyebv, zeros=zeros)


def kernel(x, w_qkv, w_dw, w_proj, temperature, _trace=False):
    from concourse.bass_utils import run_bass_kernel_spmd

    if "nc" not in _CACHE:
        _CACHE["nc"] = _build()
    nc = _CACHE["nc"]

    consts = _host_consts(w_qkv, w_dw, w_proj, temperature)
    xr = np.ascontiguousarray(np.asarray(x, dtype=np.float32).reshape(NCORES, C, HW))
    in_maps = []
    for b in range(NCORES):
        m = {"x": xr[b]}
        m.update(consts)
        in_maps.append(m)

    try:
        br = run_bass_kernel_spmd(nc, in_maps, core_ids=list(range(NCORES)),
                                  trace=_trace)
    except ModuleNotFoundError:
        br = run_bass_kernel_spmd(nc, in_maps, core_ids=list(range(NCORES)),
                                  trace=False)
    out = np.stack([r["out"] for r in br.results], axis=0).reshape(NCORES, C, H, W)
    _CACHE["last_results"] = br
    return out



# revision 27
# speedup vs baseline: 1.3468x; 1.3468x over previous
"""Trainium2 Bass kernel for Restormer-style transposed (channel) attention.

Per-core (1 of 8 batch elements), fused direct-conv design:
  The 1x1 qkv conv and the depthwise 3x3 are folded into one 3x3 full conv
  (W3[o,c,dy,dx] = w_dw[o,dy,dx] * w_qkv[o,c]) evaluated as 9 accumulated
  fp8 DoubleRow matmuls per output tile, reading a host-padded fp8 image
  (192 input channels as 2 DR planes of 96). Per-column power-of-2 weight
  scales keep fp8 mantissas busy; the scales cancel in the q/k l2-norms and
  are folded into the attention matrix for v.
  q/k conv output lands directly in 32-aligned per-head slots; per 2-row
  tile it is copied to fp8, PE-transposed, and accumulated into the gram
  via DoubleRow (2 pixel-row planes). v stays in SBUF as bf16.
  softmax over the normalized gram; proj fold: M = W_proj @ blockdiag(A);
  out = M @ v in bf16.
"""
import numpy as np

NUM_HEADS = 8
C = 192
H = W = 128
HW = H * W
C3 = 3 * C            # 576
CD = C // NUM_HEADS   # 24
NCORES = 8
EPS = 1e-12

PW = 130              # padded row width (1 zero col each side)
NB = 8                # x row-bands
BR = 16               # image rows per band
BROWS = BR + 2        # band rows incl halo
BLEN = BROWS * PW + 2 # band plane length (+1 elem guard each end)
TPB = BR // 2         # 2-row conv tiles per band
SX = 16.0             # x fp8 scale

TAPS = [(dy, dx) for dy in (-1, 0, 1) for dx in (-1, 0, 1)]


def _slot_channel(j, m):
    """qkv channel for slot m of qk sweep j (None for pad slots)."""
    g, loc = m // 32, m % 32
    if loc >= 24:
        return None
    return [48 * j, 192 + 48 * j, 48 * j + 24, 192 + 48 * j + 24][g] + loc


_CACHE = {}


def _build():
    import concourse.bass as bass
    import concourse.mybir as mybir
    import concourse.tile as tile
    from concourse import bacc
    from contextlib import ExitStack

    dt = mybir.dt
    A = mybir.AluOpType
    AF = mybir.ActivationFunctionType
    AX = mybir.AxisListType
    DR = mybir.MatmulPerfMode.DoubleRow
    f32, bf16, f8 = dt.float32, dt.bfloat16, dt.float8e4

    nc = bacc.Bacc("TRN2", num_devices=NCORES)

    xbd = [nc.dram_tensor(f"xb{b}", [96, 2 * BLEN], f8, kind="ExternalInput").ap()
           for b in range(NB)]
    xrbd = [nc.dram_tensor(f"xrb{b}", [96, 2 * BLEN], f8, kind="ExternalInput").ap()
            for b in range(NB)]
    wqkd = nc.dram_tensor("wqk", [96, 4 * 9 * 256], f8, kind="ExternalInput").ap()
    wvd = nc.dram_tensor("wv", [96, 2 * 9 * 256], f8, kind="ExternalInput").ap()
    wvrd = nc.dram_tensor("wvr", [96, 2 * 9 * 256], f8, kind="ExternalInput").ap()
    eybd = nc.dram_tensor("eyb", [128, 128], bf16, kind="ExternalInput").ap()
    mskd = nc.dram_tensor("gmask", [128, 512], f32, kind="ExternalInput").ap()
    tmpd = nc.dram_tensor("tmap", [128, 4], f32, kind="ExternalInput").ap()
    eyed = nc.dram_tensor("eye", [128, 128], f32, kind="ExternalInput").ap()
    wp0d = nc.dram_tensor("wp0", [96, C], f32, kind="ExternalInput").ap()
    wp1d = nc.dram_tensor("wp1", [96, C], f32, kind="ExternalInput").ap()
    vscd = nc.dram_tensor("vsc", [CD, C], f32, kind="ExternalInput").ap()
    qscld = nc.dram_tensor("qscl", [128, 4], f32, kind="ExternalInput").ap()
    outd = nc.dram_tensor("out", [C, HW], f32, kind="ExternalOutput").ap()

    with tile.TileContext(nc) as tc:
      with ExitStack() as _es:
        cpool = _es.enter_context(tc.tile_pool(name="const", bufs=1))
        xpool = _es.enter_context(tc.tile_pool(name="xin", bufs=1))
        qspool = _es.enter_context(tc.tile_pool(name="qs", bufs=4))
        qkpool = _es.enter_context(tc.tile_pool(name="qki", bufs=4))
        vpool = _es.enter_context(tc.tile_pool(name="vst", bufs=1))
        mpool = _es.enter_context(tc.tile_pool(name="sm", bufs=2))
        apool = _es.enter_context(tc.tile_pool(name="abd", bufs=1))
        opool = _es.enter_context(tc.tile_pool(name="outs", bufs=3))
        psC = _es.enter_context(tc.tile_pool(name="psC", bufs=3, space="PSUM"))
        psT = _es.enter_context(tc.tile_pool(name="psT", bufs=2, space="PSUM"))
        psG = _es.enter_context(tc.tile_pool(name="psG", bufs=1, space="PSUM"))
        psM = _es.enter_context(tc.tile_pool(name="psM", bufs=2, space="PSUM"))
        if True:
            # ---------- constants (scalar/ACT queue, ordered by first use) ----------
            wqk = cpool.tile([96, 4 * 9 * 256], f8, tag="wqk")
            nc.scalar.dma_start(wqk[:, 0:9 * 256], wqkd[:, 0:9 * 256])
            eyb = cpool.tile([128, 128], bf16, tag="eyb")
            nc.scalar.dma_start(eyb[:, :], eybd[:, :])
            qscl = cpool.tile([128, 4], f32, tag="qscl")
            nc.scalar.dma_start(qscl[:, :], qscld[:, :])
            for j in range(1, 4):
                nc.scalar.dma_start(wqk[:, j * 2304:(j + 1) * 2304],
                                    wqkd[:, j * 2304:(j + 1) * 2304])
            wv = cpool.tile([96, 2 * 9 * 256], f8, tag="wv")
            nc.scalar.dma_start(wv[:, :], wvd[:, :])
            wvr = cpool.tile([96, 2 * 9 * 256], f8, tag="wvr")
            nc.scalar.dma_start(wvr[:, :], wvrd[:, :])
            xrbt = []
            for b in range(NB):
                xrt = xpool.tile([96, 2 * BLEN], f8, tag=f"xrb{b}")
                nc.scalar.dma_start(xrt[:, :], xrbd[b][:, :])
                xrbt.append(xrt)
            msk = cpool.tile([128, 512], f32, tag="msk")
            nc.scalar.dma_start(msk[:, :], mskd[:, :])
            tmap = cpool.tile([128, 4], f32, tag="tmap")
            nc.scalar.dma_start(tmap[:, :], tmpd[:, :])
            eye = cpool.tile([128, 128], f32, tag="eye")
            nc.scalar.dma_start(eye[:, :], eyed[:, :])
            wp0 = cpool.tile([96, C], f32, tag="wp0")
            nc.scalar.dma_start(wp0[:, :], wp0d[:, :])
            wp1 = cpool.tile([96, C], f32, tag="wp1")
            nc.scalar.dma_start(wp1[:, :], wp1d[:, :])
            vsc = cpool.tile([CD, C], f32, tag="vsc")
            nc.scalar.dma_start(vsc[:, :], vscd[:, :])

            # x bands on the sync queue from t=0
            xbt = []
            for b in range(NB):
                xt = xpool.tile([96, 2 * BLEN], f8, tag=f"xb{b}")
                nc.sync.dma_start(xt[:, :], xbd[b][:, :])
                xbt.append(xt)

            vA = vpool.tile([128, HW], bf16, tag="vA")
            vB = vpool.tile([64, HW], bf16, tag="vB")
            gram = psG.tile([128, 512], f32, tag="g")

            # ---------- pass 1: fused conv sweeps ----------
            for b in range(NB):
                xv = xbt[b][:, :].rearrange("p (two n) -> p two n", two=2)
                # qk sweeps
                for j in range(4):
                    for t in range(TPB):
                        base = 1 + (2 * t + 1) * PW
                        pc = psC.tile([128, 2 * PW], f32, tag="pc")
                        for ti, (dy, dx) in enumerate(TAPS):
                            off = base + dy * PW + dx
                            wview = wqk[:, (j * 9 + ti) * 256:(j * 9 + ti + 1) * 256] \
                                .rearrange("p (two m) -> p two m", two=2)
                            nc.tensor.matmul(pc[:, :], wview,
                                             xv[:, :, off:off + 2 * PW],
                                             start=(ti == 0), stop=(ti == 8),
                                             perf_mode=DR)
                        pcv = pc[:, :].rearrange("p (r w) -> p r w", w=PW)
                        qs = qspool.tile([128, 256], bf16, tag="qs")
                        if (j + t) % 2 == 0:
                            nc.scalar.activation(
                                qs[:, :].rearrange("p (r w) -> p r w", w=W),
                                pcv[:, :, 1:1 + W], AF.Copy,
                                scale=qscl[0:128, j:j + 1])
                        else:
                            nc.vector.tensor_scalar_mul(
                                qs[:, :].rearrange("p (r w) -> p r w", w=W),
                                pcv[:, :, 1:1 + W], qscl[0:128, j:j + 1])
                        pt = psT.tile([128, 256], bf16, tag="pt")
                        nc.tensor.transpose(pt[:, 0:128], qs[:, 0:128], eyb[:, :])
                        nc.tensor.transpose(pt[:, 128:256], qs[:, 128:256], eyb[:, :])
                        qki = qkpool.tile([128, 256], f8, tag="qki")
                        ev2 = nc.vector.tensor_copy if (j + t) % 2 == 0 else nc.scalar.copy
                        ev2(qki[:, :], pt[:, :])
                        qk2 = qki[:, :].rearrange("p (two m) -> p two m", two=2)
                        nc.tensor.matmul(gram[:, 128 * j:128 * (j + 1)], qk2, qk2,
                                         start=(b == 0 and t == 0),
                                         stop=(b == NB - 1 and t == TPB - 1),
                                         perf_mode=DR, skip_group_check=True)
                # v sweeps: base + x-residual + W-residual passes, one psum group
                xrv = xrbt[b][:, :].rearrange("p (two n) -> p two n", two=2)
                for s in range(2):
                    nout = 128 if s == 0 else 64
                    vdst = vA if s == 0 else vB
                    for t in range(TPB):
                        base = 1 + (2 * t + 1) * PW
                        pv = psC.tile([nout, 2 * PW], f32, tag="pc")
                        ki = 0
                        for wt, xw in ((wv, xv), (wv, xrv), (wvr, xv)):
                            for ti, (dy, dx) in enumerate(TAPS):
                                off = base + dy * PW + dx
                                wview = wt[:, (s * 9 + ti) * 256:(s * 9 + ti + 1) * 256] \
                                    .rearrange("p (two m) -> p two m", two=2)[:, :, 0:nout]
                                nc.tensor.matmul(pv[:, :], wview,
                                                 xw[:, :, off:off + 2 * PW],
                                                 start=(ki == 0), stop=(ki == 26),
                                                 perf_mode=DR)
                                ki += 1
                        pvv = pv[:, :].rearrange("p (r w) -> p r w", w=PW)
                        px0 = (BR * b + 2 * t) * W
                        ev = nc.vector.tensor_copy if (s + t) % 2 == 0 else nc.scalar.copy
                        ev(vdst[:, px0:px0 + 256].rearrange("p (r w) -> p r w", w=W),
                           pvv[:, :, 1:1 + W])

            # ---------- norms ----------
            gm = mpool.tile([128, 512], f32, tag="gm")
            nc.vector.tensor_tensor(gm[:, :], gram[:, :], msk[:, :], A.mult)
            s_sb = mpool.tile([128, 4], f32, tag="ssb")
            nc.vector.tensor_reduce(s_sb[:, :],
                                    gm[:, :].rearrange("p (g c) -> p g c", g=4),
                                    AX.X, A.add)
            ns = mpool.tile([128, 4], f32, tag="ns")
            nc.scalar.sqrt(ns[:, :], s_sb[:, :])
            nsc = mpool.tile([128, 4], f32, tag="nsc")
            nc.vector.tensor_scalar_max(nsc[:, :], ns[:, :], EPS)
            ry = mpool.tile([128, 4], f32, tag="ry")
            nc.vector.reciprocal(ry[:, :], nsc[:, :])
            t1 = mpool.tile([128, 4], f32, tag="t1")
            nc.vector.tensor_tensor(t1[:, :], s_sb[:, :], ry[:, :], A.mult)
            t2 = mpool.tile([128, 4], f32, tag="t2")
            nc.vector.tensor_add(t2[:, :], nsc[:, :], t1[:, :])
            ns2 = mpool.tile([128, 4], f32, tag="ns2")
            nc.vector.tensor_scalar_mul(ns2[:, :], t2[:, :], 0.5)
            ns3 = mpool.tile([128, 4], f32, tag="ns3")
            nc.vector.tensor_scalar_max(ns3[:, :], ns2[:, :], EPS)
            rn = mpool.tile([128, 4], f32, tag="rn")
            nc.vector.reciprocal(rn[:, :], ns3[:, :])
            rkt = mpool.tile([128, 4], f32, tag="rkt")
            nc.vector.tensor_tensor(rkt[:, :], rn[:, :], tmap[:, :], A.mult)
            rq = mpool.tile([24, 8], f32, tag="rq")
            nc.sync.dma_start(rq[0:24, 0:7:2], rn[0:24, 0:4])
            nc.sync.dma_start(rq[0:24, 1:8:2], rn[64:88, 0:4])

            # ---------- softmax + A blockdiag (v scales folded in) ----------
            a0 = apool.tile([96, C], f32, tag="a0")
            a1 = apool.tile([96, C], f32, tag="a1")
            nc.vector.memset(a0[:, :], 0.0)
            nc.vector.memset(a1[:, :], 0.0)
            bt = mpool.tile([128, 8 * CD], f32, tag="bt")
            for h in range(NUM_HEADS):
                p = h // 2
                if h % 2 == 0:
                    kbase, qcol = 32, 0
                else:
                    kbase, qcol = 96, 64
                nc.vector.tensor_scalar_mul(
                    bt[kbase:kbase + CD, CD * h:CD * (h + 1)],
                    gram[kbase:kbase + CD, 128 * p + qcol:128 * p + qcol + CD],
                    rkt[kbase:kbase + CD, p:p + 1])
                ptr = psM.tile([CD, CD], f32, tag="pm")
                nc.tensor.transpose(ptr[:, :],
                                    bt[kbase:kbase + CD, CD * h:CD * (h + 1)],
                                    eye[kbase:kbase + CD, kbase:kbase + CD],
                                    tile_position=(kbase, 0))
                ls = mpool.tile([CD, CD], f32, tag="ls")
                nc.vector.tensor_scalar_mul(ls[:, :], ptr[:, :], rq[0:24, h:h + 1])
                mx = mpool.tile([CD, 1], f32, tag="mx")
                nc.vector.tensor_reduce(mx[:, :], ls[:, :], AX.X, A.max)
                mxn = mpool.tile([CD, 1], f32, tag="mxn")
                nc.vector.tensor_scalar_mul(mxn[:, :], mx[:, :], -1.0)
                es = mpool.tile([CD, CD], f32, tag="es")
                se = mpool.tile([CD, 1], f32, tag="se")
                nc.scalar.activation(es[:, :], ls[:, :], AF.Exp,
                                     bias=mxn[0:CD, 0:1], scale=1.0,
                                     accum_out=se[:, :])
                rse = mpool.tile([CD, 1], f32, tag="rse")
                nc.vector.reciprocal(rse[:, :], se[:, :])
                ah = mpool.tile([CD, CD], f32, tag="ah")
                nc.vector.tensor_scalar_mul(ah[:, :], es[:, :], rse[0:CD, 0:1])
                ah2 = mpool.tile([CD, CD], f32, tag="ah2")
                nc.vector.tensor_tensor(ah2[:, :], ah[:, :],
                                        vsc[0:CD, CD * h:CD * (h + 1)], A.mult)
                adst = a0 if h < 4 else a1
                r0 = 24 * (h % 4)
                nc.sync.dma_start(adst[r0:r0 + CD, CD * h:CD * (h + 1)], ah2[:, :])

            # ---------- M^T = A_bd^T @ W_proj^T ----------
            mt0 = cpool.tile([128, C], bf16, tag="mt0")
            mt1 = cpool.tile([64, C], bf16, tag="mt1")
            pmt0 = psM.tile([128, C], f32, tag="pm")
            nc.tensor.matmul(pmt0[:, :], a0[:, 0:128], wp0[:, :],
                             start=True, stop=False)
            nc.tensor.matmul(pmt0[:, :], a1[:, 0:128], wp1[:, :],
                             start=False, stop=True)
            nc.scalar.copy(mt0[:, :], pmt0[:, :])
            pmt1 = psM.tile([64, C], f32, tag="pm")
            nc.tensor.matmul(pmt1[:, :], a0[:, 128:192], wp0[:, :],
                             start=True, stop=False)
            nc.tensor.matmul(pmt1[:, :], a1[:, 128:192], wp1[:, :],
                             start=False, stop=True)
            nc.scalar.copy(mt1[:, :], pmt1[:, :])

            # ---------- pass 2: out = M @ v ----------
            for b0 in range(0, HW, 512):
                pa = psM.tile([128, 512], f32, tag="pm")
                nc.tensor.matmul(pa[:, :], mt0[:, 0:128], vA[:, b0:b0 + 512],
                                 start=True, stop=False)
                nc.tensor.matmul(pa[:, :], mt1[:, 0:128], vB[:, b0:b0 + 512],
                                 start=False, stop=True)
                oa = opool.tile([128, 512], f32, tag="oa")
                nc.scalar.copy(oa[:, :], pa[:, :])
                nc.scalar.dma_start(outd[0:128, b0:b0 + 512], oa[:, :])
                pb = psM.tile([64, 512], f32, tag="pm")
                nc.tensor.matmul(pb[:, :], mt0[:, 128:192], vA[:, b0:b0 + 512],
                                 start=True, stop=False)
                nc.tensor.matmul(pb[:, :], mt1[:, 128:192], vB[:, b0:b0 + 512],
                                 start=False, stop=True)
                ob = opool.tile([64, 512], f32, tag="ob")
                nc.vector.tensor_copy(ob[:, :], pb[:, :])
                nc.sync.dma_start(outd[128:192, b0:b0 + 512], ob[:, :])

    nc.compile()
    return nc


def _host_inputs(x, w_qkv, w_dw, w_proj, temperature):
    import ml_dtypes
    F8 = ml_dtypes.float8_e4m3

    xin = np.asarray(x, np.float32).reshape(NCORES, C, H, W)
    pad = np.zeros((NCORES, C, H + 2, PW), np.float32)
    pad[:, :, 1:1 + H, 1:1 + W] = xin
    x8 = (pad * SX).astype(F8)                       # [8, 192, 130, 130]
    xr8 = (pad * SX - x8.astype(np.float32)).astype(F8)
    bands, rbands = [], []
    for src, dst in ((x8, bands), (xr8, rbands)):
        for b in range(NB):
            seg = src[:, :, BR * b:BR * b + BROWS, :].reshape(NCORES, C, BROWS * PW)
            buf = np.zeros((NCORES, C, BLEN), F8)
            buf[:, :, 1:1 + BROWS * PW] = seg
            dst.append(np.concatenate([buf[:, 0:96], buf[:, 96:192]], axis=2))

    wq = np.asarray(w_qkv, np.float32)               # [576, 192]
    wd = np.asarray(w_dw, np.float32).reshape(C3, 3, 3)
    mxw = np.abs(wd).max(axis=(1, 2)) * np.abs(wq).max(axis=1)
    scol = 2.0 ** np.floor(np.log2(192.0 / np.maximum(mxw, 1e-30)))
    # fused[o, t, c] = wd[o, t] * wq[o, c] * scol[o]
    fw = (wd.reshape(C3, 9, 1) * wq.reshape(C3, 1, C)
          * scol.reshape(C3, 1, 1)).astype(np.float32)

    wqk = np.zeros((96, 4 * 9 * 256), np.float32)
    for j in range(4):
        slots = [(m, _slot_channel(j, m)) for m in range(128)]
        slots = [(m, o) for m, o in slots if o is not None]
        ms = np.array([m for m, _ in slots])
        os_ = np.array([o for _, o in slots])
        for ti in range(9):
            blk = (j * 9 + ti) * 256
            wqk[:, blk + ms] = fw[os_, ti, 0:96].T
            wqk[:, blk + 128 + ms] = fw[os_, ti, 96:192].T
    wvv = np.zeros((96, 2 * 9 * 256), np.float32)
    for s in range(2):
        nout = 128 if s == 0 else 64
        os_ = 384 + 128 * s + np.arange(nout)
        for ti in range(9):
            blk = (s * 9 + ti) * 256
            wvv[:, blk:blk + nout] = fw[os_, ti, 0:96].T
            wvv[:, blk + 128:blk + 128 + nout] = fw[os_, ti, 96:192].T
    wv8 = wvv.astype(F8)
    wvr8 = (wvv - wv8.astype(np.float32)).astype(F8)

    wpT = np.ascontiguousarray(np.asarray(w_proj, np.float32).T)  # [192, 192]
    gmask = np.tile(np.eye(128, dtype=np.float32), (1, 4))
    tmapv = np.ones((128, 4), dtype=np.float32)
    tf = np.asarray(temperature, dtype=np.float32).reshape(-1)
    for p in range(4):
        tmapv[32:56, p] = tf[2 * p]
        tmapv[96:120, p] = tf[2 * p + 1]
    vscv = np.zeros((CD, C), np.float32)
    vscv[:, :] = (1.0 / (SX * scol[384:576])).reshape(1, C)
    qsclv = np.ones((128, 4), np.float32)
    for j in range(4):
        for m in range(128):
            o = _slot_channel(j, m)
            if o is not None:
                qsclv[m, j] = 1.0 / (SX * scol[o])

    consts = dict(
        wqk=wqk.astype(F8), wv=wv8, wvr=wvr8,
        eyb=np.eye(128).astype(ml_dtypes.bfloat16),
        gmask=gmask, tmap=tmapv, eye=np.eye(128, dtype=np.float32),
        wp0=wpT[0:96].copy(), wp1=wpT[96:192].copy(), vsc=vscv, qscl=qsclv)
    return consts, bands, rbands


def kernel(x, w_qkv, w_dw, w_proj, temperature, _trace=False):
    from concourse.bass_utils import run_bass_kernel_spmd

    if "nc" not in _CACHE:
        _CACHE["nc"] = _build()
    nc = _CACHE["nc"]

    consts, bands, rbands = _host_inputs(x, w_qkv, w_dw, w_proj, temperature)
    in_maps = []
    for core in range(NCORES):
        m = dict(consts)
        for b in range(NB):
            m[f"xb{b}"] = np.ascontiguousarray(bands[b][core])
            m[f"xrb{b}"] = np.ascontiguousarray(rbands[b][core])
        in_maps.append(m)

    try:
        br = run_bass_kernel_spmd(nc, in_maps, core_ids=list(range(NCORES)),
                                  trace=_trace)
    except ModuleNotFoundError:
        br = run_bass_kernel_spmd(nc, in_maps, core_ids=list(range(NCORES)),
                                  trace=False)
    out = np.stack([r["out"] for r in br.results], axis=0).reshape(NCORES, C, H, W)
    _CACHE["last_results"] = br
    return out


# revision 34
# speedup vs baseline: 1.4322x; 1.0635x over previous
"""Trainium2 Bass kernel for Restormer-style transposed (channel) attention.

Per-core (1 of 8 batch elements), fused direct-conv design:
  The 1x1 qkv conv and the depthwise 3x3 are folded into one 3x3 full conv
  (W3[o,c,dy,dx] = w_dw[o,dy,dx] * w_qkv[o,c]) evaluated as 9 accumulated
  fp8 DoubleRow matmuls per output tile, reading a host-padded fp8 image
  (192 input channels as 2 DR planes of 96). Per-column power-of-2 weight
  scales keep fp8 mantissas busy; the scales cancel in the q/k l2-norms and
  are folded into the attention matrix for v.
  q/k conv output lands directly in 32-aligned per-head slots; per 2-row
  tile it is copied to fp8, PE-transposed, and accumulated into the gram
  via DoubleRow (2 pixel-row planes). v stays in SBUF as bf16.
  softmax over the normalized gram; proj fold: M = W_proj @ blockdiag(A);
  out = M @ v in bf16.
"""
import numpy as np

NUM_HEADS = 8
C = 192
H = W = 128
HW = H * W
C3 = 3 * C            # 576
CD = C // NUM_HEADS   # 24
NCORES = 8
EPS = 1e-12

PW = 130              # padded row width (1 zero col each side)
NB = 8                # x row-bands
BR = 16               # image rows per band
BROWS = BR + 2        # band rows incl halo
BLEN = BROWS * PW + 2 # band plane length (+1 elem guard each end)
TPB = BR // 2         # 2-row conv tiles per band
SX = 16.0             # x fp8 scale

TAPS = [(dy, dx) for dy in (-1, 0, 1) for dx in (-1, 0, 1)]


def _slot_channel(j, m):
    """qkv channel for slot m of qk sweep j (None for pad slots)."""
    g, loc = m // 32, m % 32
    if loc >= 24:
        return None
    return [48 * j, 192 + 48 * j, 48 * j + 24, 192 + 48 * j + 24][g] + loc


_CACHE = {}


def _build():
    import concourse.bass as bass
    import concourse.mybir as mybir
    import concourse.tile as tile
    from concourse import bacc
    from contextlib import ExitStack

    dt = mybir.dt
    A = mybir.AluOpType
    AF = mybir.ActivationFunctionType
    AX = mybir.AxisListType
    DR = mybir.MatmulPerfMode.DoubleRow
    f32, bf16, f8 = dt.float32, dt.bfloat16, dt.float8e4

    nc = bacc.Bacc("TRN2", num_devices=NCORES)

    xbd = [nc.dram_tensor(f"xb{b}", [96, 2 * BLEN], f8, kind="ExternalInput").ap()
           for b in range(NB)]
    xrbd = [nc.dram_tensor(f"xrb{b}", [96, 2 * BLEN], f8, kind="ExternalInput").ap()
            for b in range(NB)]
    wqkd = nc.dram_tensor("wqk", [96, 4 * 9 * 256], f8, kind="ExternalInput").ap()
    wvd = nc.dram_tensor("wv", [96, 2 * 9 * 256], f8, kind="ExternalInput").ap()
    wvrd = nc.dram_tensor("wvr", [96, 2 * 9 * 256], f8, kind="ExternalInput").ap()
    eybd = nc.dram_tensor("eyb", [128, 128], bf16, kind="ExternalInput").ap()
    mskd = nc.dram_tensor("gmask", [128, 512], f32, kind="ExternalInput").ap()
    tmpd = nc.dram_tensor("tmap", [128, 4], f32, kind="ExternalInput").ap()
    eyed = nc.dram_tensor("eye", [128, 128], f32, kind="ExternalInput").ap()
    wp0d = nc.dram_tensor("wp0", [96, C], f32, kind="ExternalInput").ap()
    wp1d = nc.dram_tensor("wp1", [96, C], f32, kind="ExternalInput").ap()
    vscd = nc.dram_tensor("vsc", [CD, C], f32, kind="ExternalInput").ap()
    qscld = nc.dram_tensor("qscl", [128, 4], f32, kind="ExternalInput").ap()
    outd = nc.dram_tensor("out", [C, HW], f32, kind="ExternalOutput").ap()

    with tile.TileContext(nc) as tc:
      with ExitStack() as _es:
        cpool = _es.enter_context(tc.tile_pool(name="const", bufs=1))
        xpool = _es.enter_context(tc.tile_pool(name="xin", bufs=1))
        qspool = _es.enter_context(tc.tile_pool(name="qs", bufs=4))
        qkpool = _es.enter_context(tc.tile_pool(name="qki", bufs=4))
        vpool = _es.enter_context(tc.tile_pool(name="vst", bufs=1))
        mpool = _es.enter_context(tc.tile_pool(name="sm", bufs=2))
        apool = _es.enter_context(tc.tile_pool(name="abd", bufs=1))
        opool = _es.enter_context(tc.tile_pool(name="outs", bufs=3))
        psC = _es.enter_context(tc.tile_pool(name="psC", bufs=3, space="PSUM"))
        psT = _es.enter_context(tc.tile_pool(name="psT", bufs=2, space="PSUM"))
        psG = _es.enter_context(tc.tile_pool(name="psG", bufs=1, space="PSUM"))
        psM = _es.enter_context(tc.tile_pool(name="psM", bufs=2, space="PSUM"))
        if True:
            # ---------- inputs, all on the SP (sync) queue in first-need order
            # (keeps ACT's sequencer free for copies from t=0) ----------
            xbt = [None] * NB
            xt0 = xpool.tile([96, 2 * BLEN], f8, tag="xb0")
            nc.sync.dma_start(xt0[:, :], xbd[0][:, :])
            xbt[0] = xt0
            wqk = cpool.tile([96, 4 * 9 * 256], f8, tag="wqk")
            nc.sync.dma_start(wqk[:, 0:9 * 256], wqkd[:, 0:9 * 256])
            eyb = cpool.tile([128, 128], bf16, tag="eyb")
            nc.sync.dma_start(eyb[:, :], eybd[:, :])
            qscl = cpool.tile([128, 4], f32, tag="qscl")
            nc.sync.dma_start(qscl[:, :], qscld[:, :])
            for j in range(1, 4):
                nc.sync.dma_start(wqk[:, j * 2304:(j + 1) * 2304],
                                  wqkd[:, j * 2304:(j + 1) * 2304])
            for b in range(1, NB):
                xt = xpool.tile([96, 2 * BLEN], f8, tag=f"xb{b}")
                nc.sync.dma_start(xt[:, :], xbd[b][:, :])
                xbt[b] = xt
            msk = cpool.tile([128, 512], f32, tag="msk")
            nc.sync.dma_start(msk[:, :], mskd[:, :])
            tmap = cpool.tile([128, 4], f32, tag="tmap")
            nc.sync.dma_start(tmap[:, :], tmpd[:, :])
            eye = cpool.tile([128, 128], f32, tag="eye")
            nc.sync.dma_start(eye[:, :], eyed[:, :])
            vsc = cpool.tile([CD, C], f32, tag="vsc")
            nc.sync.dma_start(vsc[:, :], vscd[:, :])
            wv = cpool.tile([96, 2 * 9 * 256], f8, tag="wv")
            nc.sync.dma_start(wv[:, :], wvd[:, :])
            wvr = cpool.tile([96, 2 * 9 * 256], f8, tag="wvr")
            nc.sync.dma_start(wvr[:, :], wvrd[:, :])
            xrbt = []
            for b in range(NB):
                xrt = xpool.tile([96, 2 * BLEN], f8, tag=f"xrb{b}")
                nc.sync.dma_start(xrt[:, :], xrbd[b][:, :])
                xrbt.append(xrt)
            wp0 = cpool.tile([96, C], f32, tag="wp0")
            nc.sync.dma_start(wp0[:, :], wp0d[:, :])
            wp1 = cpool.tile([96, C], f32, tag="wp1")
            nc.sync.dma_start(wp1[:, :], wp1d[:, :])

            vA = vpool.tile([128, HW], bf16, tag="vA")
            vB = vpool.tile([64, HW], bf16, tag="vB")
            gram = psG.tile([128, 512], f32, tag="g")

            # ---------- pass 1a: fused conv, qk sweeps (sweep-major) ----------
            for j in range(4):
                for b in range(NB):
                    xv = xbt[b][:, :].rearrange("p (two n) -> p two n", two=2)
                    for t in range(TPB):
                        base = 1 + (2 * t + 1) * PW
                        pc = psC.tile([128, 2 * PW], f32, tag="pc")
                        for ti, (dy, dx) in enumerate(TAPS):
                            off = base + dy * PW + dx
                            wview = wqk[:, (j * 9 + ti) * 256:(j * 9 + ti + 1) * 256] \
                                .rearrange("p (two m) -> p two m", two=2)
                            nc.tensor.matmul(pc[:, :], wview,
                                             xv[:, :, off:off + 2 * PW],
                                             start=(ti == 0), stop=(ti == 8),
                                             perf_mode=DR)
                        pcv = pc[:, :].rearrange("p (r w) -> p r w", w=PW)
                        qs = qspool.tile([128, 256], bf16, tag="qs")
                        if (j + t) % 2 == 0:
                            nc.scalar.activation(
                                qs[:, :].rearrange("p (r w) -> p r w", w=W),
                                pcv[:, :, 1:1 + W], AF.Copy,
                                scale=qscl[0:128, j:j + 1])
                        else:
                            nc.vector.tensor_scalar_mul(
                                qs[:, :].rearrange("p (r w) -> p r w", w=W),
                                pcv[:, :, 1:1 + W], qscl[0:128, j:j + 1])
                        pt = psT.tile([128, 256], bf16, tag="pt")
                        nc.tensor.transpose(pt[:, 0:128], qs[:, 0:128], eyb[:, :])
                        nc.tensor.transpose(pt[:, 128:256], qs[:, 128:256], eyb[:, :])
                        qki = qkpool.tile([128, 256], f8, tag="qki")
                        ev2 = nc.vector.tensor_copy if (j + t) % 2 == 0 else nc.scalar.copy
                        ev2(qki[:, :], pt[:, :])
                        qk2 = qki[:, :].rearrange("p (two m) -> p two m", two=2)
                        nc.tensor.matmul(gram[:, 128 * j:128 * (j + 1)], qk2, qk2,
                                         start=(b == 0 and t == 0),
                                         stop=(b == NB - 1 and t == TPB - 1),
                                         perf_mode=DR, skip_group_check=True)
            # ---------- norms ----------
            gm = mpool.tile([128, 512], f32, tag="gm")
            nc.vector.tensor_tensor(gm[:, :], gram[:, :], msk[:, :], A.mult)
            s_sb = mpool.tile([128, 4], f32, tag="ssb")
            nc.vector.tensor_reduce(s_sb[:, :],
                                    gm[:, :].rearrange("p (g c) -> p g c", g=4),
                                    AX.X, A.add)
            ns = mpool.tile([128, 4], f32, tag="ns")
            nc.scalar.sqrt(ns[:, :], s_sb[:, :])
            nsc = mpool.tile([128, 4], f32, tag="nsc")
            nc.vector.tensor_scalar_max(nsc[:, :], ns[:, :], EPS)
            ry = mpool.tile([128, 4], f32, tag="ry")
            nc.vector.reciprocal(ry[:, :], nsc[:, :])
            t1 = mpool.tile([128, 4], f32, tag="t1")
            nc.vector.tensor_tensor(t1[:, :], s_sb[:, :], ry[:, :], A.mult)
            t2 = mpool.tile([128, 4], f32, tag="t2")
            nc.vector.tensor_add(t2[:, :], nsc[:, :], t1[:, :])
            ns2 = mpool.tile([128, 4], f32, tag="ns2")
            nc.vector.tensor_scalar_mul(ns2[:, :], t2[:, :], 0.5)
            ns3 = mpool.tile([128, 4], f32, tag="ns3")
            nc.vector.tensor_scalar_max(ns3[:, :], ns2[:, :], EPS)
            rn = mpool.tile([128, 4], f32, tag="rn")
            nc.vector.reciprocal(rn[:, :], ns3[:, :])
            rkt = mpool.tile([128, 4], f32, tag="rkt")
            nc.vector.tensor_tensor(rkt[:, :], rn[:, :], tmap[:, :], A.mult)
            rq = mpool.tile([24, 8], f32, tag="rq")
            nc.sync.dma_start(rq[0:24, 0:7:2], rn[0:24, 0:4])
            nc.sync.dma_start(rq[0:24, 1:8:2], rn[64:88, 0:4])

            # ---------- softmax + A blockdiag (v scales folded in) ----------
            a0 = apool.tile([96, C], f32, tag="a0")
            a1 = apool.tile([96, C], f32, tag="a1")
            nc.vector.memset(a0[:, :], 0.0)
            nc.vector.memset(a1[:, :], 0.0)
            bt = mpool.tile([128, 8 * CD], f32, tag="bt")
            for h in range(NUM_HEADS):
                p = h // 2
                if h % 2 == 0:
                    kbase, qcol = 32, 0
                else:
                    kbase, qcol = 96, 64
                nc.vector.tensor_scalar_mul(
                    bt[kbase:kbase + CD, CD * h:CD * (h + 1)],
                    gram[kbase:kbase + CD, 128 * p + qcol:128 * p + qcol + CD],
                    rkt[kbase:kbase + CD, p:p + 1])
                ptr = psM.tile([CD, CD], f32, tag="pm")
                nc.tensor.transpose(ptr[:, :],
                                    bt[kbase:kbase + CD, CD * h:CD * (h + 1)],
                                    eye[kbase:kbase + CD, kbase:kbase + CD],
                                    tile_position=(kbase, 0))
                ls = mpool.tile([CD, CD], f32, tag="ls")
                nc.vector.tensor_scalar_mul(ls[:, :], ptr[:, :], rq[0:24, h:h + 1])
                mx = mpool.tile([CD, 1], f32, tag="mx")
                nc.vector.tensor_reduce(mx[:, :], ls[:, :], AX.X, A.max)
                mxn = mpool.tile([CD, 1], f32, tag="mxn")
                nc.vector.tensor_scalar_mul(mxn[:, :], mx[:, :], -1.0)
                es = mpool.tile([CD, CD], f32, tag="es")
                se = mpool.tile([CD, 1], f32, tag="se")
                nc.scalar.activation(es[:, :], ls[:, :], AF.Exp,
                                     bias=mxn[0:CD, 0:1], scale=1.0,
                                     accum_out=se[:, :])
                rse = mpool.tile([CD, 1], f32, tag="rse")
                nc.vector.reciprocal(rse[:, :], se[:, :])
                ah = mpool.tile([CD, CD], f32, tag="ah")
                nc.vector.tensor_scalar_mul(ah[:, :], es[:, :], rse[0:CD, 0:1])
                ah2 = mpool.tile([CD, CD], f32, tag="ah2")
                nc.vector.tensor_tensor(ah2[:, :], ah[:, :],
                                        vsc[0:CD, CD * h:CD * (h + 1)], A.mult)
                adst = a0 if h < 4 else a1
                r0 = 24 * (h % 4)
                nc.sync.dma_start(adst[r0:r0 + CD, CD * h:CD * (h + 1)], ah2[:, :])

            # ---------- pass 1b: v conv sweeps (overlap norms/softmax above) ----
            # base + x-residual + W-residual passes, one psum group per tile
            for s in range(2):
                nout = 128 if s == 0 else 64
                vdst = vA if s == 0 else vB
                for b in range(NB):
                    xv = xbt[b][:, :].rearrange("p (two n) -> p two n", two=2)
                    xrv = xrbt[b][:, :].rearrange("p (two n) -> p two n", two=2)
                    for t in range(TPB):
                        base = 1 + (2 * t + 1) * PW
                        pv = psC.tile([nout, 2 * PW], f32, tag="pc")
                        ki = 0
                        for wt, xw in ((wv, xv), (wv, xrv), (wvr, xv)):
                            for ti, (dy, dx) in enumerate(TAPS):
                                off = base + dy * PW + dx
                                wview = wt[:, (s * 9 + ti) * 256:(s * 9 + ti + 1) * 256] \
                                    .rearrange("p (two m) -> p two m", two=2)[:, :, 0:nout]
                                nc.tensor.matmul(pv[:, :], wview,
                                                 xw[:, :, off:off + 2 * PW],
                                                 start=(ki == 0), stop=(ki == 26),
                                                 perf_mode=DR)
                                ki += 1
                        pvv = pv[:, :].rearrange("p (r w) -> p r w", w=PW)
                        px0 = (BR * b + 2 * t) * W
                        ev = nc.vector.tensor_copy if (s + t) % 2 == 0 else nc.scalar.copy
                        ev(vdst[:, px0:px0 + 256].rearrange("p (r w) -> p r w", w=W),
                           pvv[:, :, 1:1 + W])

            # ---------- M^T = A_bd^T @ W_proj^T ----------
            mt0 = cpool.tile([128, C], bf16, tag="mt0")
            mt1 = cpool.tile([64, C], bf16, tag="mt1")
            pmt0 = psM.tile([128, C], f32, tag="pm")
            nc.tensor.matmul(pmt0[:, :], a0[:, 0:128], wp0[:, :],
                             start=True, stop=False)
            nc.tensor.matmul(pmt0[:, :], a1[:, 0:128], wp1[:, :],
                             start=False, stop=True)
            nc.scalar.copy(mt0[:, :], pmt0[:, :])
            pmt1 = psM.tile([64, C], f32, tag="pm")
            nc.tensor.matmul(pmt1[:, :], a0[:, 128:192], wp0[:, :],
                             start=True, stop=False)
            nc.tensor.matmul(pmt1[:, :], a1[:, 128:192], wp1[:, :],
                             start=False, stop=True)
            nc.scalar.copy(mt1[:, :], pmt1[:, :])

            # ---------- pass 2: out = M @ v ----------
            for b0 in range(0, HW, 512):
                pa = psC.tile([128, 512], f32, tag="pc")
                nc.tensor.matmul(pa[:, :], mt0[:, 0:128], vA[:, b0:b0 + 512],
                                 start=True, stop=False)
                nc.tensor.matmul(pa[:, :], mt1[:, 0:128], vB[:, b0:b0 + 512],
                                 start=False, stop=True)
                oa = opool.tile([128, 512], f32, tag="oa")
                nc.scalar.copy(oa[:, :], pa[:, :])
                nc.scalar.dma_start(outd[0:128, b0:b0 + 512], oa[:, :])
                pb = psM.tile([64, 512], f32, tag="pm")
                nc.tensor.matmul(pb[:, :], mt0[:, 128:192], vA[:, b0:b0 + 512],
                                 start=True, stop=False)
                nc.tensor.matmul(pb[:, :], mt1[:, 128:192], vB[:, b0:b0 + 512],
                                 start=False, stop=True)
                ob = opool.tile([64, 512], f32, tag="ob")
                nc.vector.tensor_copy(ob[:, :], pb[:, :])
                nc.sync.dma_start(outd[128:192, b0:b0 + 512], ob[:, :])

    nc.compile()
    return nc


def _host_inputs(x, w_qkv, w_dw, w_proj, temperature):
    import ml_dtypes
    F8 = ml_dtypes.float8_e4m3

    xin = np.asarray(x, np.float32).reshape(NCORES, C, H, W)
    pad = np.zeros((NCORES, C, H + 2, PW), np.float32)
    pad[:, :, 1:1 + H, 1:1 + W] = xin
    x8 = (pad * SX).astype(F8)                       # [8, 192, 130, 130]
    xr8 = (pad * SX - x8.astype(np.float32)).astype(F8)
    bands, rbands = [], []
    for src, dst in ((x8, bands), (xr8, rbands)):
        for b in range(NB):
            seg = src[:, :, BR * b:BR * b + BROWS, :].reshape(NCORES, C, BROWS * PW)
            buf = np.zeros((NCORES, C, BLEN), F8)
            buf[:, :, 1:1 + BROWS * PW] = seg
            dst.append(np.concatenate([buf[:, 0:96], buf[:, 96:192]], axis=2))

    wq = np.asarray(w_qkv, np.float32)               # [576, 192]
    wd = np.asarray(w_dw, np.float32).reshape(C3, 3, 3)
    mxw = np.abs(wd).max(axis=(1, 2)) * np.abs(wq).max(axis=1)
    scol = 2.0 ** np.floor(np.log2(192.0 / np.maximum(mxw, 1e-30)))
    # fused[o, t, c] = wd[o, t] * wq[o, c] * scol[o]
    fw = (wd.reshape(C3, 9, 1) * wq.reshape(C3, 1, C)
          * scol.reshape(C3, 1, 1)).astype(np.float32)

    wqk = np.zeros((96, 4 * 9 * 256), np.float32)
    for j in range(4):
        slots = [(m, _slot_channel(j, m)) for m in range(128)]
        slots = [(m, o) for m, o in slots if o is not None]
        ms = np.array([m for m, _ in slots])
        os_ = np.array([o for _, o in slots])
        for ti in range(9):
            blk = (j * 9 + ti) * 256
            wqk[:, blk + ms] = fw[os_, ti, 0:96].T
            wqk[:, blk + 128 + ms] = fw[os_, ti, 96:192].T
    wvv = np.zeros((96, 2 * 9 * 256), np.float32)
    for s in range(2):
        nout = 128 if s == 0 else 64
        os_ = 384 + 128 * s + np.arange(nout)
        for ti in range(9):
            blk = (s * 9 + ti) * 256
            wvv[:, blk:blk + nout] = fw[os_, ti, 0:96].T
            wvv[:, blk + 128:blk + 128 + nout] = fw[os_, ti, 96:192].T
    wv8 = wvv.astype(F8)
    wvr8 = (wvv - wv8.astype(np.float32)).astype(F8)

    wpT = np.ascontiguousarray(np.asarray(w_proj, np.float32).T)  # [192, 192]
    gmask = np.tile(np.eye(128, dtype=np.float32), (1, 4))
    tmapv = np.ones((128, 4), dtype=np.float32)
    tf = np.asarray(temperature, dtype=np.float32).reshape(-1)
    for p in range(4):
        tmapv[32:56, p] = tf[2 * p]
        tmapv[96:120, p] = tf[2 * p + 1]
    vscv = np.zeros((CD, C), np.float32)
    vscv[:, :] = (1.0 / (SX * scol[384:576])).reshape(1, C)
    qsclv = np.ones((128, 4), np.float32)
    for j in range(4):
        for m in range(128):
            o = _slot_channel(j, m)
            if o is not None:
                qsclv[m, j] = 1.0 / (SX * scol[o])

    consts = dict(
        wqk=wqk.astype(F8), wv=wv8, wvr=wvr8,
        eyb=np.eye(128).astype(ml_dtypes.bfloat16),
        gmask=gmask, tmap=tmapv, eye=np.eye(128, dtype=np.float32),
        wp0=wpT[0:96].copy(), wp1=wpT[96:192].copy(), vsc=vscv, qscl=qsclv)
    return consts, bands, rbands


def kernel(x, w_qkv, w_dw, w_proj, temperature, _trace=False):
    from concourse.bass_utils import run_bass_kernel_spmd

    if "nc" not in _CACHE:
        _CACHE["nc"] = _build()
    nc = _CACHE["nc"]

    consts, bands, rbands = _host_inputs(x, w_qkv, w_dw, w_proj, temperature)
    in_maps = []
    for core in range(NCORES):
        m = dict(consts)
        for b in range(NB):
            m[f"xb{b}"] = np.ascontiguousarray(bands[b][core])
            m[f"xrb{b}"] = np.ascontiguousarray(rbands[b][core])
        in_maps.append(m)

    try:
        br = run_bass_kernel_spmd(nc, in_maps, core_ids=list(range(NCORES)),
                                  trace=_trace)
    except ModuleNotFoundError:
        br = run_bass_kernel_spmd(nc, in_maps, core_ids=list(range(NCORES)),
                                  trace=False)
    out = np.stack([r["out"] for r in br.results], axis=0).reshape(NCORES, C, H, W)
    _CACHE["last_results"] = br
    return out


# revision 37
# speedup vs baseline: 1.5001x; 1.0473x over previous
"""Trainium2 Bass kernel for Restormer-style transposed (channel) attention.

Per-core (1 of 8 batch elements), fused direct-conv design:
  The 1x1 qkv conv and the depthwise 3x3 are folded into one 3x3 full conv
  (W3[o,c,dy,dx] = w_dw[o,dy,dx] * w_qkv[o,c]) evaluated as 9 accumulated
  fp8 DoubleRow matmuls per output tile, reading a host-padded fp8 image
  (192 input channels as 2 DR planes of 96). Per-column power-of-2 weight
  scales keep fp8 mantissas busy; the scales cancel in the q/k l2-norms and
  are folded into the attention matrix for v.
  q/k conv output lands directly in 32-aligned per-head slots; per 2-row
  tile it is copied to fp8, PE-transposed, and accumulated into the gram
  via DoubleRow (2 pixel-row planes). v stays in SBUF as bf16.
  softmax over the normalized gram; proj fold: M = W_proj @ blockdiag(A);
  out = M @ v in bf16.
"""
import numpy as np

NUM_HEADS = 8
C = 192
H = W = 128
HW = H * W
C3 = 3 * C            # 576
CD = C // NUM_HEADS   # 24
NCORES = 8
EPS = 1e-12

PW = 130              # padded row width (1 zero col each side)
NB = 8                # x row-bands
BR = 16               # image rows per band
BROWS = BR + 2        # band rows incl halo
BLEN = BROWS * PW + 2 # band plane length (+1 elem guard each end)
TPB = BR // 2         # 2-row conv tiles per band
SX = 16.0             # x fp8 scale

TAPS = [(dy, dx) for dy in (-1, 0, 1) for dx in (-1, 0, 1)]


def _slot_channel(j, m):
    """qkv channel for slot m of qk sweep j (None for pad slots)."""
    g, loc = m // 32, m % 32
    if loc >= 24:
        return None
    return [48 * j, 192 + 48 * j, 48 * j + 24, 192 + 48 * j + 24][g] + loc


_CACHE = {}


def _build():
    import concourse.bass as bass
    import concourse.mybir as mybir
    import concourse.tile as tile
    from concourse import bacc
    from contextlib import ExitStack

    dt = mybir.dt
    A = mybir.AluOpType
    AF = mybir.ActivationFunctionType
    AX = mybir.AxisListType
    DR = mybir.MatmulPerfMode.DoubleRow
    f32, bf16, f8 = dt.float32, dt.bfloat16, dt.float8e4

    nc = bacc.Bacc("TRN2", num_devices=NCORES)

    xbd = [nc.dram_tensor(f"xb{b}", [96, 2 * BLEN], f8, kind="ExternalInput").ap()
           for b in range(NB)]
    xrbd = [nc.dram_tensor(f"xrb{b}", [96, 2 * BLEN], f8, kind="ExternalInput").ap()
            for b in range(NB)]
    wqkd = nc.dram_tensor("wqk", [96, 4 * 9 * 256], f8, kind="ExternalInput").ap()
    wvd = nc.dram_tensor("wv", [96, 2 * 9 * 256], f8, kind="ExternalInput").ap()
    wvrd = nc.dram_tensor("wvr", [96, 2 * 9 * 256], f8, kind="ExternalInput").ap()
    eybd = nc.dram_tensor("eyb", [128, 128], bf16, kind="ExternalInput").ap()
    mskd = nc.dram_tensor("gmask", [128, 512], f32, kind="ExternalInput").ap()
    tmpd = nc.dram_tensor("tmap", [128, 4], f32, kind="ExternalInput").ap()
    eyed = nc.dram_tensor("eye", [128, 128], f32, kind="ExternalInput").ap()
    wp0d = nc.dram_tensor("wp0", [96, C], f32, kind="ExternalInput").ap()
    wp1d = nc.dram_tensor("wp1", [96, C], f32, kind="ExternalInput").ap()
    vscd = nc.dram_tensor("vsc", [CD, C], f32, kind="ExternalInput").ap()
    qscld = nc.dram_tensor("qscl", [128, 4], f32, kind="ExternalInput").ap()
    outd = nc.dram_tensor("out", [C, HW], f32, kind="ExternalOutput").ap()

    with tile.TileContext(nc) as tc:
      with ExitStack() as _es:
        cpool = _es.enter_context(tc.tile_pool(name="const", bufs=1))
        xpool = _es.enter_context(tc.tile_pool(name="xin", bufs=1))
        qspool = _es.enter_context(tc.tile_pool(name="qs", bufs=4))
        qkpool = _es.enter_context(tc.tile_pool(name="qki", bufs=4))
        vpool = _es.enter_context(tc.tile_pool(name="vst", bufs=1))
        mpool = _es.enter_context(tc.tile_pool(name="sm", bufs=2))
        apool = _es.enter_context(tc.tile_pool(name="abd", bufs=1))
        opool = _es.enter_context(tc.tile_pool(name="outs", bufs=3))
        psC = _es.enter_context(tc.tile_pool(name="psC", bufs=3, space="PSUM"))
        psT = _es.enter_context(tc.tile_pool(name="psT", bufs=2, space="PSUM"))
        psG = _es.enter_context(tc.tile_pool(name="psG", bufs=1, space="PSUM"))
        psM = _es.enter_context(tc.tile_pool(name="psM", bufs=2, space="PSUM"))
        if True:
            # ---------- inputs, all on the SP (sync) queue in first-need order
            # (keeps ACT's sequencer free for copies from t=0) ----------
            xbt = [None] * NB
            xt0 = xpool.tile([96, 2 * BLEN], f8, tag="xb0")
            nc.sync.dma_start(xt0[:, :], xbd[0][:, :])
            xbt[0] = xt0
            wqk = cpool.tile([96, 4 * 9 * 256], f8, tag="wqk")
            nc.sync.dma_start(wqk[:, 0:9 * 256], wqkd[:, 0:9 * 256])
            eyb = cpool.tile([128, 128], bf16, tag="eyb")
            nc.sync.dma_start(eyb[:, :], eybd[:, :])
            qscl = cpool.tile([128, 4], f32, tag="qscl")
            nc.sync.dma_start(qscl[:, :], qscld[:, :])
            for j in range(1, 4):
                nc.sync.dma_start(wqk[:, j * 2304:(j + 1) * 2304],
                                  wqkd[:, j * 2304:(j + 1) * 2304])
            for b in range(1, NB):
                xt = xpool.tile([96, 2 * BLEN], f8, tag=f"xb{b}")
                nc.sync.dma_start(xt[:, :], xbd[b][:, :])
                xbt[b] = xt
            msk = cpool.tile([128, 512], f32, tag="msk")
            nc.sync.dma_start(msk[:, :], mskd[:, :])
            tmap = cpool.tile([128, 4], f32, tag="tmap")
            nc.sync.dma_start(tmap[:, :], tmpd[:, :])
            eye = cpool.tile([128, 128], f32, tag="eye")
            nc.sync.dma_start(eye[:, :], eyed[:, :])
            vsc = cpool.tile([CD, C], f32, tag="vsc")
            nc.sync.dma_start(vsc[:, :], vscd[:, :])
            wv = cpool.tile([96, 2 * 9 * 256], f8, tag="wv")
            nc.sync.dma_start(wv[:, :], wvd[:, :])
            wvr = cpool.tile([96, 2 * 9 * 256], f8, tag="wvr")
            nc.sync.dma_start(wvr[:, :], wvrd[:, :])
            xrbt = []
            for b in range(NB):
                xrt = xpool.tile([96, 2 * BLEN], f8, tag=f"xrb{b}")
                nc.sync.dma_start(xrt[:, :], xrbd[b][:, :])
                xrbt.append(xrt)
            wp0 = cpool.tile([96, C], f32, tag="wp0")
            nc.sync.dma_start(wp0[:, :], wp0d[:, :])
            wp1 = cpool.tile([96, C], f32, tag="wp1")
            nc.sync.dma_start(wp1[:, :], wp1d[:, :])

            vA = vpool.tile([128, HW], bf16, tag="vA")
            vB = vpool.tile([64, HW], bf16, tag="vB")
            gram = psG.tile([128, 512], f32, tag="g")

            # ---------- pass 1a: fused conv, qk sweeps (sweep-major) ----------
            for j in range(4):
                for b in range(NB):
                    xv = xbt[b][:, :].rearrange("p (two n) -> p two n", two=2)
                    for t in range(TPB):
                        base = 1 + (2 * t + 1) * PW
                        pc = psC.tile([128, 2 * PW], f32, tag="pc")
                        for ti, (dy, dx) in enumerate(TAPS):
                            off = base + dy * PW + dx
                            wview = wqk[:, (j * 9 + ti) * 256:(j * 9 + ti + 1) * 256] \
                                .rearrange("p (two m) -> p two m", two=2)
                            nc.tensor.matmul(pc[:, :], wview,
                                             xv[:, :, off:off + 2 * PW],
                                             start=(ti == 0), stop=(ti == 8),
                                             perf_mode=DR)
                        pcv = pc[:, :].rearrange("p (r w) -> p r w", w=PW)
                        qs = qspool.tile([128, 256], bf16, tag="qs")
                        if (j + t) % 2 == 0:
                            nc.scalar.activation(
                                qs[:, :].rearrange("p (r w) -> p r w", w=W),
                                pcv[:, :, 1:1 + W], AF.Copy,
                                scale=qscl[0:128, j:j + 1])
                        else:
                            nc.vector.tensor_scalar_mul(
                                qs[:, :].rearrange("p (r w) -> p r w", w=W),
                                pcv[:, :, 1:1 + W], qscl[0:128, j:j + 1])
                        pt = psT.tile([128, 256], bf16, tag="pt")
                        nc.tensor.transpose(pt[:, 0:128], qs[:, 0:128], eyb[:, :])
                        nc.tensor.transpose(pt[:, 128:256], qs[:, 128:256], eyb[:, :])
                        qki = qkpool.tile([128, 256], f8, tag="qki")
                        ev2 = nc.vector.tensor_copy if (j + t) % 2 == 0 else nc.scalar.copy
                        ev2(qki[:, :], pt[:, :])
                        qk2 = qki[:, :].rearrange("p (two m) -> p two m", two=2)
                        nc.tensor.matmul(gram[:, 128 * j:128 * (j + 1)], qk2, qk2,
                                         start=(b == 0 and t == 0),
                                         stop=(b == NB - 1 and t == TPB - 1),
                                         perf_mode=DR, skip_group_check=True)
            # ---------- norms ----------
            gm = mpool.tile([128, 512], f32, tag="gm")
            nc.vector.tensor_tensor(gm[:, :], gram[:, :], msk[:, :], A.mult)
            s_sb = mpool.tile([128, 4], f32, tag="ssb")
            nc.vector.tensor_reduce(s_sb[:, :],
                                    gm[:, :].rearrange("p (g c) -> p g c", g=4),
                                    AX.X, A.add)
            ns = mpool.tile([128, 4], f32, tag="ns")
            nc.scalar.sqrt(ns[:, :], s_sb[:, :])
            nsc = mpool.tile([128, 4], f32, tag="nsc")
            nc.vector.tensor_scalar_max(nsc[:, :], ns[:, :], EPS)
            ry = mpool.tile([128, 4], f32, tag="ry")
            nc.vector.reciprocal(ry[:, :], nsc[:, :])
            t1 = mpool.tile([128, 4], f32, tag="t1")
            nc.vector.tensor_tensor(t1[:, :], s_sb[:, :], ry[:, :], A.mult)
            t2 = mpool.tile([128, 4], f32, tag="t2")
            nc.vector.tensor_add(t2[:, :], nsc[:, :], t1[:, :])
            ns2 = mpool.tile([128, 4], f32, tag="ns2")
            nc.vector.tensor_scalar_mul(ns2[:, :], t2[:, :], 0.5)
            ns3 = mpool.tile([128, 4], f32, tag="ns3")
            nc.vector.tensor_scalar_max(ns3[:, :], ns2[:, :], EPS)
            rn = mpool.tile([128, 4], f32, tag="rn")
            nc.vector.reciprocal(rn[:, :], ns3[:, :])
            rkt = mpool.tile([128, 4], f32, tag="rkt")
            nc.vector.tensor_tensor(rkt[:, :], rn[:, :], tmap[:, :], A.mult)
            rq = mpool.tile([24, 8], f32, tag="rq")
            nc.sync.dma_start(rq[0:24, 0:7:2], rn[0:24, 0:4])
            nc.sync.dma_start(rq[0:24, 1:8:2], rn[64:88, 0:4])

            # ---------- softmax + A blockdiag (v scales folded in) ----------
            a0 = apool.tile([96, C], f32, tag="a0")
            a1 = apool.tile([96, C], f32, tag="a1")
            nc.vector.memset(a0[:, :], 0.0)
            nc.vector.memset(a1[:, :], 0.0)
            bt = mpool.tile([128, 8 * CD], f32, tag="bt")
            for h in range(NUM_HEADS):
                p = h // 2
                if h % 2 == 0:
                    kbase, qcol = 32, 0
                else:
                    kbase, qcol = 96, 64
                nc.vector.tensor_scalar_mul(
                    bt[kbase:kbase + CD, CD * h:CD * (h + 1)],
                    gram[kbase:kbase + CD, 128 * p + qcol:128 * p + qcol + CD],
                    rkt[kbase:kbase + CD, p:p + 1])
                ptr = psM.tile([CD, CD], f32, tag="pm")
                nc.tensor.transpose(ptr[:, :],
                                    bt[kbase:kbase + CD, CD * h:CD * (h + 1)],
                                    eye[kbase:kbase + CD, kbase:kbase + CD],
                                    tile_position=(kbase, 0))
                ls = mpool.tile([CD, CD], f32, tag="ls")
                nc.vector.tensor_scalar_mul(ls[:, :], ptr[:, :], rq[0:24, h:h + 1])
                mx = mpool.tile([CD, 1], f32, tag="mx")
                nc.vector.tensor_reduce(mx[:, :], ls[:, :], AX.X, A.max)
                mxn = mpool.tile([CD, 1], f32, tag="mxn")
                nc.vector.tensor_scalar_mul(mxn[:, :], mx[:, :], -1.0)
                es = mpool.tile([CD, CD], f32, tag="es")
                se = mpool.tile([CD, 1], f32, tag="se")
                nc.scalar.activation(es[:, :], ls[:, :], AF.Exp,
                                     bias=mxn[0:CD, 0:1], scale=1.0,
                                     accum_out=se[:, :])
                rse = mpool.tile([CD, 1], f32, tag="rse")
                nc.vector.reciprocal(rse[:, :], se[:, :])
                ah = mpool.tile([CD, CD], f32, tag="ah")
                nc.vector.tensor_scalar_mul(ah[:, :], es[:, :], rse[0:CD, 0:1])
                ah2 = mpool.tile([CD, CD], f32, tag="ah2")
                nc.vector.tensor_tensor(ah2[:, :], ah[:, :],
                                        vsc[0:CD, CD * h:CD * (h + 1)], A.mult)
                adst = a0 if h < 4 else a1
                r0 = 24 * (h % 4)
                nc.sync.dma_start(adst[r0:r0 + CD, CD * h:CD * (h + 1)], ah2[:, :])

            # ---------- M^T = A_bd^T @ W_proj^T ----------
            mt0 = cpool.tile([128, C], bf16, tag="mt0")
            mt1 = cpool.tile([64, C], bf16, tag="mt1")
            pmt0 = psM.tile([128, C], f32, tag="pm")
            nc.tensor.matmul(pmt0[:, :], a0[:, 0:128], wp0[:, :],
                             start=True, stop=False)
            nc.tensor.matmul(pmt0[:, :], a1[:, 0:128], wp1[:, :],
                             start=False, stop=True)
            nc.scalar.copy(mt0[:, :], pmt0[:, :])
            pmt1 = psM.tile([64, C], f32, tag="pm")
            nc.tensor.matmul(pmt1[:, :], a0[:, 128:192], wp0[:, :],
                             start=True, stop=False)
            nc.tensor.matmul(pmt1[:, :], a1[:, 128:192], wp1[:, :],
                             start=False, stop=True)
            nc.scalar.copy(mt1[:, :], pmt1[:, :])

            # ---------- pass 1b: v conv sweeps fused with pass-2 output ------
            # per band: conv v (base + x-res + W-res into one psum group),
            # then immediately out = M @ v for that band's pixels
            for b in range(NB):
                xv = xbt[b][:, :].rearrange("p (two n) -> p two n", two=2)
                xrv = xrbt[b][:, :].rearrange("p (two n) -> p two n", two=2)
                for s in range(2):
                    nout = 128 if s == 0 else 64
                    vdst = vA if s == 0 else vB
                    for t in range(TPB):
                        base = 1 + (2 * t + 1) * PW
                        pv = psC.tile([nout, 2 * PW], f32, tag="pc")
                        ki = 0
                        for wt, xw in ((wv, xv), (wv, xrv), (wvr, xv)):
                            for ti, (dy, dx) in enumerate(TAPS):
                                off = base + dy * PW + dx
                                wview = wt[:, (s * 9 + ti) * 256:(s * 9 + ti + 1) * 256] \
                                    .rearrange("p (two m) -> p two m", two=2)[:, :, 0:nout]
                                nc.tensor.matmul(pv[:, :], wview,
                                                 xw[:, :, off:off + 2 * PW],
                                                 start=(ki == 0), stop=(ki == 26),
                                                 perf_mode=DR)
                                ki += 1
                        pvv = pv[:, :].rearrange("p (r w) -> p r w", w=PW)
                        px0 = (BR * b + 2 * t) * W
                        ev = nc.vector.tensor_copy if (s + t) % 2 == 0 else nc.scalar.copy
                        ev(vdst[:, px0:px0 + 256].rearrange("p (r w) -> p r w", w=W),
                           pvv[:, :, 1:1 + W])
                for b0 in range(BR * b * W, (BR * b + BR) * W, 512):
                    pa = psC.tile([128, 512], f32, tag="pc")
                    nc.tensor.matmul(pa[:, :], mt0[:, 0:128], vA[:, b0:b0 + 512],
                                     start=True, stop=False)
                    nc.tensor.matmul(pa[:, :], mt1[:, 0:128], vB[:, b0:b0 + 512],
                                     start=False, stop=True)
                    oa = opool.tile([128, 512], f32, tag="oa")
                    nc.scalar.copy(oa[:, :], pa[:, :])
                    nc.scalar.dma_start(outd[0:128, b0:b0 + 512], oa[:, :])
                    pb = psM.tile([64, 512], f32, tag="pm")
                    nc.tensor.matmul(pb[:, :], mt0[:, 128:192], vA[:, b0:b0 + 512],
                                     start=True, stop=False)
                    nc.tensor.matmul(pb[:, :], mt1[:, 128:192], vB[:, b0:b0 + 512],
                                     start=False, stop=True)
                    ob = opool.tile([64, 512], f32, tag="ob")
                    nc.vector.tensor_copy(ob[:, :], pb[:, :])
                    nc.sync.dma_start(outd[128:192, b0:b0 + 512], ob[:, :])

    nc.compile()
    return nc


def _host_inputs(x, w_qkv, w_dw, w_proj, temperature):
    import ml_dtypes
    F8 = ml_dtypes.float8_e4m3

    xin = np.asarray(x, np.float32).reshape(NCORES, C, H, W)
    pad = np.zeros((NCORES, C, H + 2, PW), np.float32)
    pad[:, :, 1:1 + H, 1:1 + W] = xin
    x8 = (pad * SX).astype(F8)                       # [8, 192, 130, 130]
    xr8 = (pad * SX - x8.astype(np.float32)).astype(F8)
    bands, rbands = [], []
    for src, dst in ((x8, bands), (xr8, rbands)):
        for b in range(NB):
            seg = src[:, :, BR * b:BR * b + BROWS, :].reshape(NCORES, C, BROWS * PW)
            buf = np.zeros((NCORES, C, BLEN), F8)
            buf[:, :, 1:1 + BROWS * PW] = seg
            dst.append(np.concatenate([buf[:, 0:96], buf[:, 96:192]], axis=2))

    wq = np.asarray(w_qkv, np.float32)               # [576, 192]
    wd = np.asarray(w_dw, np.float32).reshape(C3, 3, 3)
    mxw = np.abs(wd).max(axis=(1, 2)) * np.abs(wq).max(axis=1)
    scol = 2.0 ** np.floor(np.log2(192.0 / np.maximum(mxw, 1e-30)))
    # fused[o, t, c] = wd[o, t] * wq[o, c] * scol[o]
    fw = (wd.reshape(C3, 9, 1) * wq.reshape(C3, 1, C)
          * scol.reshape(C3, 1, 1)).astype(np.float32)

    wqk = np.zeros((96, 4 * 9 * 256), np.float32)
    for j in range(4):
        slots = [(m, _slot_channel(j, m)) for m in range(128)]
        slots = [(m, o) for m, o in slots if o is not None]
        ms = np.array([m for m, _ in slots])
        os_ = np.array([o for _, o in slots])
        for ti in range(9):
            blk = (j * 9 + ti) * 256
            wqk[:, blk + ms] = fw[os_, ti, 0:96].T
            wqk[:, blk + 128 + ms] = fw[os_, ti, 96:192].T
    wvv = np.zeros((96, 2 * 9 * 256), np.float32)
    for s in range(2):
        nout = 128 if s == 0 else 64
        os_ = 384 + 128 * s + np.arange(nout)
        for ti in range(9):
            blk = (s * 9 + ti) * 256
            wvv[:, blk:blk + nout] = fw[os_, ti, 0:96].T
            wvv[:, blk + 128:blk + 128 + nout] = fw[os_, ti, 96:192].T
    wv8 = wvv.astype(F8)
    wvr8 = (wvv - wv8.astype(np.float32)).astype(F8)

    wpT = np.ascontiguousarray(np.asarray(w_proj, np.float32).T)  # [192, 192]
    gmask = np.tile(np.eye(128, dtype=np.float32), (1, 4))
    tmapv = np.ones((128, 4), dtype=np.float32)
    tf = np.asarray(temperature, dtype=np.float32).reshape(-1)
    for p in range(4):
        tmapv[32:56, p] = tf[2 * p]
        tmapv[96:120, p] = tf[2 * p + 1]
    vscv = np.zeros((CD, C), np.float32)
    vscv[:, :] = (1.0 / (SX * scol[384:576])).reshape(1, C)
    qsclv = np.ones((128, 4), np.float32)
    for j in range(4):
        for m in range(128):
            o = _slot_channel(j, m)
            if o is not None:
                qsclv[m, j] = 1.0 / (SX * scol[o])

    consts = dict(
        wqk=wqk.astype(F8), wv=wv8, wvr=wvr8,
        eyb=np.eye(128).astype(ml_dtypes.bfloat16),
        gmask=gmask, tmap=tmapv, eye=np.eye(128, dtype=np.float32),
        wp0=wpT[0:96].copy(), wp1=wpT[96:192].copy(), vsc=vscv, qscl=qsclv)
    return consts, bands, rbands


def kernel(x, w_qkv, w_dw, w_proj, temperature, _trace=False):
    from concourse.bass_utils import run_bass_kernel_spmd

    if "nc" not in _CACHE:
        _CACHE["nc"] = _build()
    nc = _CACHE["nc"]

    consts, bands, rbands = _host_inputs(x, w_qkv, w_dw, w_proj, temperature)
    in_maps = []
    for core in range(NCORES):
        m = dict(consts)
        for b in range(NB):
            m[f"xb{b}"] = np.ascontiguousarray(bands[b][core])
            m[f"xrb{b}"] = np.ascontiguousarray(rbands[b][core])
        in_maps.append(m)

    try:
        br = run_bass_kernel_spmd(nc, in_maps, core_ids=list(range(NCORES)),
                                  trace=_trace)
    except ModuleNotFoundError:
        br = run_bass_kernel_spmd(nc, in_maps, core_ids=list(range(NCORES)),
                                  trace=False)
    out = np.stack([r["out"] for r in br.results], axis=0).reshape(NCORES, C, H, W)
    _CACHE["last_results"] = br
    return out
